# revision 22
# baseline (speedup 1.0000x reference)
"""DLRM forward (bottom MLP + 26-table EmbeddingBag + dot interaction + top MLP)
on 8 Trainium2 NeuronCores via Bass/Tile.

Sharding: batch-parallel. Each core handles 1024 of the 8192 samples and owns a
replicated copy of all 26 embedding tables in its HBM. No collectives.

Gather strategy: `dma_gather` (SWDGE) needs int16 row indices, so each 200000-row
table is addressed as 7 chunks of <=32768 rows. Per (table, chunk) the host
routes that chunk's indices into a compacted int16 stream (bag-sorted order is
preserved). Pooling of the gathered rows into per-bag sums runs on the
TensorEngine: for each 128-row gathered tile, a one-hot assignment matrix A
(built on-chip from a host-supplied relative-bag vector via is_equal against an
iota row) maps rows -> bags of one 128-bag window, accumulating in PSUM.

The SPMD program must be identical across cores, so tile counts per
(table, chunk) are padded to the max over the 8 cores and each tile emits
pooling matmuls for the union of bag-windows any core touches there; a core's
`bagrel` data zeroes the windows it does not use.
"""

import os
import sys
import time

import numpy as np

# ---------------------------------------------------------------- constants
B = 8192
L = 10
NT = 26
V = 200000
D = 64
NCORE = 8
BC = B // NCORE            # 1024 samples per core
SLOT = BC * L              # 10240 gathers per (core, table)
SCH = 32768                # chunk rows (int16-addressable)
NCH = (V + SCH - 1) // SCH  # 7
TILE = 128
TP = int(os.environ.get("DLRM_TP", "16"))  # max tiles per gather piece
JB = 16                    # A-matrix build batch (jobs)
NWIN = BC // TILE          # 8 bag windows per core
PAD_BAGREL = -512.0
SINGLE_PACKET = os.environ.get("DLRM_SINGLE_PACKET", "0") == "1"
HOST_A = os.environ.get("DLRM_HOST_A", "0") == "1"
NEGPAD = os.environ.get("DLRM_NEGPAD", "0") == "1"
AC = int(os.environ.get("DLRM_AC", "120"))  # host-A jobs per DMA chunk


# ---------------------------------------------------------------- host prep
def _prep(lS_i):
    """Compute the shared static structure + per-core device arrays."""
    lS = np.asarray(lS_i).astype(np.int64)

    seg_loc = {}
    seg_bag = {}
    nrows = np.zeros((NCORE, NT, NCH), np.int64)
    bag_of_pos = np.arange(SLOT, dtype=np.int64) // L
    for c in range(NCORE):
        for t in range(NT):
            idx = lS[t, c * SLOT:(c + 1) * SLOT].astype(np.int64)
            ch = idx >> 15
            order = np.argsort(ch, kind="stable")
            sidx = idx[order]
            sbag = bag_of_pos[order]
            sch = ch[order]
            bounds = np.searchsorted(sch, np.arange(NCH + 1))
            for k in range(NCH):
                lo, hi = bounds[k], bounds[k + 1]
                seg_loc[(c, t, k)] = (sidx[lo:hi] - (k << 15)).astype(np.int16)
                seg_bag[(c, t, k)] = sbag[lo:hi]
                nrows[c, t, k] = hi - lo

    T_tk = np.maximum(1, ((nrows.max(axis=0) + TILE - 1) // TILE)).astype(np.int64)

    # static walk: gather pieces + pooling jobs
    pieces = []        # (t, k, ntiles, idxcol0)
    piece_tile0 = []   # first tile index of the piece within its (t, k)
    jobs = []          # [t, w, piece_idx, tile_in_piece, start, stop]
    idxcols = 0
    first_last = {}
    for t in range(NT):
        for k in range(NCH):
            ntk = int(T_tk[t, k])
            tile0 = 0
            while tile0 < ntk:
                nt_p = min(TP, ntk - tile0)
                p_idx = len(pieces)
                pieces.append((t, k, nt_p, idxcols))
                piece_tile0.append(tile0)
                for i in range(nt_p):
                    gt = tile0 + i
                    wins = set()
                    for c in range(NCORE):
                        bags = seg_bag[(c, t, k)][gt * TILE:(gt + 1) * TILE]
                        if len(bags):
                            wins.update(np.unique(bags // TILE).tolist())
                    if not wins:
                        wins = {0}
                    for w in sorted(wins):
                        j = len(jobs)
                        jobs.append([t, int(w), p_idx, i, False, False])
                        if (t, w) not in first_last:
                            first_last[(t, w)] = [j, j]
                        else:
                            first_last[(t, w)][1] = j
                idxcols += nt_p * (TILE // 16)
                tile0 += nt_p
    for t in range(NT):
        for w in range(NWIN):
            assert (t, w) in first_last, (t, w)
    for (t, w), (f, l_) in first_last.items():
        jobs[f][4] = True
        jobs[l_][5] = True
    njobs = len(jobs)

    pad16 = np.int16(-1) if NEGPAD else np.int16(0)
    idx16 = np.zeros((NCORE, 16, idxcols), np.int16)
    nreal = np.zeros((NCORE, len(pieces)), np.int32)
    bagrel = np.full((NCORE, TILE, njobs), PAD_BAGREL, np.float16)
    for c in range(NCORE):
        for p_idx, (t, k, nt_p, col0) in enumerate(pieces):
            tile0 = piece_tile0[p_idx]
            loc = seg_loc[(c, t, k)]
            n = len(loc)
            lo = tile0 * TILE
            hi = min(n, (tile0 + nt_p) * TILE)
            chunk = np.full(nt_p * TILE, pad16, np.int16)
            if hi > lo:
                chunk[: hi - lo] = loc[lo:hi]
            nreal[c, p_idx] = max(hi - lo, 0)
            if NEGPAD and nreal[c, p_idx] == 0:
                chunk[0] = 0        # keep >=1 valid idx so the DMA sem fires
                nreal[c, p_idx] = 1
            # element i -> [i % 16, i // 16]
            idx16[c, :, col0:col0 + nt_p * (TILE // 16)] = chunk.reshape(-1, 16).T

        for j, (t, w, p_idx, i, _s, _e) in enumerate(jobs):
            _t, k, nt_p, col0 = pieces[p_idx]
            gt = piece_tile0[p_idx] + i
            seg = seg_bag[(c, t, k)][gt * TILE:(gt + 1) * TILE]
            if len(seg):
                bagrel[c, : len(seg), j] = (
                    seg.astype(np.float32) - 128.0 * w).astype(np.float16)

    ahost = None
    if HOST_A:
        import ml_dtypes
        # one-hot A per job from bagrel: A[p, j, b] = (bagrel[p, j] == b)
        ahost = np.zeros((NCORE, TILE, njobs, TILE), np.uint8)
        one = np.float32(1.0).astype(ml_dtypes.float8_e4m3).view(np.uint8)
        for c in range(NCORE):
            rel = bagrel[c].astype(np.int32)                  # [128, njobs]
            valid = (rel >= 0) & (rel < TILE)
            np.put_along_axis(ahost[c], rel.clip(0, TILE - 1)[..., None], one,
                              axis=2)
            ahost[c][~valid] = 0
        ahost = ahost.view(ml_dtypes.float8_e4m3)

    idx16_rep = np.tile(idx16, (1, 8, 1))      # replicate into 8 bands of 16
    static = dict(pieces=pieces, piece_tile0=piece_tile0, jobs=jobs,
                  idxcols=idxcols, njobs=njobs, T_tk=T_tk)
    return static, idx16_rep, bagrel, nreal, ahost


# ---------------------------------------------------------------- program
def _build(static):
    import concourse.tile as tile
    from concourse import bacc, mybir
    from concourse.masks import make_identity

    f32 = mybir.dt.float32
    bf16 = mybir.dt.bfloat16
    fp16 = mybir.dt.float16
    i16 = mybir.dt.int16
    i32 = mybir.dt.int32
    f8 = mybir.dt.float8e4
    AF = mybir.ActivationFunctionType
    ALU = mybir.AluOpType
    AX = mybir.AxisListType

    pieces = static["pieces"]
    jobs = static["jobs"]
    idxcols = static["idxcols"]
    njobs = static["njobs"]

    nc = bacc.Bacc("TRN2", target_bir_lowering=False, debug=False,
                   num_devices=NCORE)

    tab = nc.dram_tensor("tab", [NT, V, D], f32, kind="ExternalInput").ap()
    idx16_d = nc.dram_tensor("idx16", [128, idxcols], i16, kind="ExternalInput").ap()
    if HOST_A:
        ahost_d = nc.dram_tensor("ahost", [128, njobs, 128], f8,
                                 kind="ExternalInput").ap()
    else:
        bagrel_d = nc.dram_tensor("bagrel", [128, njobs], fp16,
                                  kind="ExternalInput").ap()
        iota_d = nc.dram_tensor("iotaf", [128, 128], fp16,
                                kind="ExternalInput").ap()
    if NEGPAD:
        nreal_d = nc.dram_tensor("nreal", [1, len(pieces)], i32,
                                 kind="ExternalInput").ap()
    xT_d = nc.dram_tensor("xT", [13, BC], f32, kind="ExternalInput").ap()
    w1t_d = nc.dram_tensor("w1t", [13, 512], f32, kind="ExternalInput").ap()
    w2t_d = nc.dram_tensor("w2t", [128, 4, 256], f32, kind="ExternalInput").ap()
    w3t_d = nc.dram_tensor("w3t", [128, 2, 64], f32, kind="ExternalInput").ap()
    b1_d = nc.dram_tensor("b1", [128, 4], f32, kind="ExternalInput").ap()
    b2_d = nc.dram_tensor("b2", [128, 2], f32, kind="ExternalInput").ap()
    b3_d = nc.dram_tensor("b3", [64, 1], f32, kind="ExternalInput").ap()
    tw1t_d = nc.dram_tensor("tw1t", [128, 4, 512], f32, kind="ExternalInput").ap()
    tw2t_d = nc.dram_tensor("tw2t", [128, 4, 256], f32, kind="ExternalInput").ap()
    tw3t_d = nc.dram_tensor("tw3t", [128, 2, 1], f32, kind="ExternalInput").ap()
    tb1_d = nc.dram_tensor("tb1", [128, 4], f32, kind="ExternalInput").ap()
    tb2_d = nc.dram_tensor("tb2", [128, 2], f32, kind="ExternalInput").ap()
    tb3_d = nc.dram_tensor("tb3", [1, 1], f32, kind="ExternalInput").ap()
    out_d = nc.dram_tensor("out", [BC, 1], f32, kind="ExternalOutput").ap()

    idx16_s = nc.alloc_sbuf_tensor("idx16_s", [128, idxcols], i16).ap()
    if not HOST_A:
        bagrel_s = nc.alloc_sbuf_tensor("bagrel_s", [128, njobs], fp16).ap()
        iota_s = nc.alloc_sbuf_tensor("iota_s", [128, 128], fp16).ap()
    if NEGPAD:
        nreal_s = nc.alloc_sbuf_tensor("nreal_s", [1, len(pieces)], i32).ap()
    tall = nc.alloc_sbuf_tensor("tall", [128, NWIN, NT + 1, D], bf16).ap()
    r_all = nc.alloc_sbuf_tensor("r_all", [128, NWIN, 416], f32).ap()
    itmp = nc.alloc_sbuf_tensor("itmp", [128, NT, D], bf16).ap()
    xT_s = nc.alloc_sbuf_tensor("xT_s", [13, BC], f32).ap()
    w1t_s = nc.alloc_sbuf_tensor("w1t_s", [13, 512], f32).ap()
    w2t_s = nc.alloc_sbuf_tensor("w2t_s", [128, 4, 256], f32).ap()
    w3t_s = nc.alloc_sbuf_tensor("w3t_s", [128, 2, 64], f32).ap()
    b1_s = nc.alloc_sbuf_tensor("b1_s", [128, 4], f32).ap()
    b2_s = nc.alloc_sbuf_tensor("b2_s", [128, 2], f32).ap()
    b3_s = nc.alloc_sbuf_tensor("b3_s", [64, 1], f32).ap()
    tw1t_s = nc.alloc_sbuf_tensor("tw1t_s", [128, 4, 512], f32).ap()
    tw2t_s = nc.alloc_sbuf_tensor("tw2t_s", [128, 4, 256], f32).ap()
    tw3t_s = nc.alloc_sbuf_tensor("tw3t_s", [128, 2, 1], f32).ap()
    tb1_s = nc.alloc_sbuf_tensor("tb1_s", [128, 4], f32).ap()
    tb2_s = nc.alloc_sbuf_tensor("tb2_s", [128, 2], f32).ap()
    tb3_s = nc.alloc_sbuf_tensor("tb3_s", [1, 1], f32).ap()
    h1t = nc.alloc_sbuf_tensor("h1t", [128, 4, BC], f32).ap()
    h2t = nc.alloc_sbuf_tensor("h2t", [128, 2, BC], f32).ap()
    h3t = nc.alloc_sbuf_tensor("h3t", [64, BC], f32).ap()
    ident = nc.alloc_sbuf_tensor("ident", [128, 128], f32).ap()
    p_s = nc.alloc_sbuf_tensor("p_s", [1, BC], f32).ap()

    with tile.TileContext(nc) as tc:
        with (
            tc.tile_pool(name="g", bufs=6) as gp,
            tc.tile_pool(name="ab", bufs=(2 if HOST_A else 3)) as abp,
            tc.tile_pool(name="gbf", bufs=4) as gbfp,
            tc.tile_pool(name="pp", bufs=2, space="PSUM") as pp,
            tc.tile_pool(name="mp", bufs=2, space="PSUM") as mp,
            tc.tile_pool(name="tp", bufs=2, space="PSUM") as tpp,
        ):
            # ---- input loads
            nc.sync.dma_start(idx16_s, idx16_d)
            if not HOST_A:
                nc.sync.dma_start(bagrel_s, bagrel_d)
                nc.sync.dma_start(iota_s, iota_d)
            if NEGPAD:
                nc.sync.dma_start(nreal_s, nreal_d)
            nc.sync.dma_start(xT_s, xT_d)
            for a, b_ in [(w1t_s, w1t_d), (w2t_s, w2t_d), (w3t_s, w3t_d),
                          (b1_s, b1_d), (b2_s, b2_d), (b3_s, b3_d),
                          (tw1t_s, tw1t_d), (tw2t_s, tw2t_d), (tw3t_s, tw3t_d),
                          (tb1_s, tb1_d), (tb2_s, tb2_d), (tb3_s, tb3_d)]:
                nc.sync.dma_start(a, b_)
            make_identity(nc, ident)
            nc.vector.memset(r_all, 0.0)
            nc.vector.memset(tall, 0.0)

            # ---- bottom MLP (no dependence on gathers; overlaps them)
            for m in range(4):
                for n2 in range(2):
                    ps = mp.tile([128, 512], f32, tag="mp")
                    nc.tensor.matmul(ps, w1t_s[:, m * 128:(m + 1) * 128],
                                     xT_s[:, n2 * 512:(n2 + 1) * 512],
                                     start=True, stop=True)
                    nc.scalar.activation(h1t[:, m, n2 * 512:(n2 + 1) * 512], ps,
                                         AF.Relu, bias=b1_s[:, m:m + 1])
            for m in range(2):
                for n2 in range(2):
                    ps = mp.tile([128, 512], f32, tag="mp")
                    for kk in range(4):
                        nc.tensor.matmul(ps, w2t_s[:, kk, m * 128:(m + 1) * 128],
                                         h1t[:, kk, n2 * 512:(n2 + 1) * 512],
                                         start=(kk == 0), stop=(kk == 3))
                    nc.scalar.activation(h2t[:, m, n2 * 512:(n2 + 1) * 512], ps,
                                         AF.Relu, bias=b2_s[:, m:m + 1])
            for n2 in range(2):
                ps = mp.tile([128, 512], f32, tag="mp")
                for kk in range(2):
                    nc.tensor.matmul(ps[0:64, :], w3t_s[:, kk, :],
                                     h2t[:, kk, n2 * 512:(n2 + 1) * 512],
                                     start=(kk == 0), stop=(kk == 1))
                nc.scalar.activation(h3t[:, n2 * 512:(n2 + 1) * 512], ps[0:64, :],
                                     AF.Relu, bias=b3_s)
            for blk in range(NWIN):
                ps = tpp.tile([128, 128], f32, tag="tp")
                nc.tensor.transpose(ps[:, 0:64],
                                    h3t[:, blk * 128:(blk + 1) * 128],
                                    ident[0:64, 0:64])
                nc.vector.tensor_copy(r_all[:, blk, 0:64], ps[:, 0:64])
                nc.scalar.activation(tall[:, blk, 0, :], ps[:, 0:64], AF.Copy)

            # ---- gather + pool
            # interaction group n: needs tall slots 0..n (slot n = table n-1);
            # emitted as soon as table n-1's pooled values land in tall so the
            # DVE work overlaps later tables' gathers instead of serializing
            # into a tail after the last gather.
            def emit_interaction(n):
                off = (n - 1) * n // 2
                for blk in range(NWIN):
                    nc.vector.tensor_tensor(
                        itmp[:, 0:n, :],
                        tall[:, blk, n, None, :].to_broadcast([128, n, 64]),
                        tall[:, blk, 0:n, :],
                        op=ALU.mult,
                    )
                    nc.vector.tensor_reduce(
                        r_all[:, blk, 64 + off:64 + off + n],
                        itmp[:, 0:n, :], axis=AX.X, op=ALU.add,
                    )

            np_lim = int(os.environ.get("DLRM_NPIECES", "1000000"))
            cnt_reg = nc.gpsimd.alloc_register("cnt") if NEGPAD else None
            if NEGPAD:
                # prime the gather buffers: skipped (-1) trailing idxs leave
                # the destination untouched, so it must hold finite floats
                # (anything*0 accumulates correctly unless it's NaN garbage)
                for _ in range(6):
                    tmpg = gp.tile([128, TP, 64], f32, tag="gbuf")
                    nc.vector.memset(tmpg, 0.0)
            job_ptr = 0
            a_tile = None
            a_base = -1
            pooled = None
            cur_t = -1
            for p_idx, (t, k, nt_p, col0) in enumerate(pieces):
                if p_idx >= np_lim:
                    job_ptr = len(jobs)
                    break
                if t != cur_t:
                    if pooled is not None:
                        nc.scalar.activation(tall[:, :, cur_t + 1, :],
                                             pooled[:], AF.Copy)
                        emit_interaction(cur_t + 1)
                    pooled = pp.tile([128, NWIN, 64], f32, tag="pp")
                    cur_t = t
                rows = min(V - k * SCH, SCH)
                src = tab[t, k * SCH:k * SCH + rows, :]
                g = gp.tile([128, TP, 64], f32, tag="gbuf")
                n_idx = nt_p * TILE
                if NEGPAD:
                    nc.gpsimd.reg_load(cnt_reg, nreal_s[0:1, p_idx:p_idx + 1])
                    cnt = cnt_reg
                else:
                    cnt = n_idx
                nc.gpsimd.dma_gather(
                    g[:, :nt_p, :], src,
                    idx16_s[:, col0:col0 + nt_p * 8],
                    num_idxs=n_idx, num_idxs_reg=cnt,
                    elem_size=64, elem_step=64, single_packet=SINGLE_PACKET,
                )
                gbf = gbfp.tile([128, TP, 64], bf16, tag="gbf")
                nc.scalar.activation(gbf[:, :nt_p, :], g[:, :nt_p, :], AF.Copy)
                while job_ptr < len(jobs) and jobs[job_ptr][2] == p_idx:
                    jt, w, _p, i, st, en = jobs[job_ptr]
                    if HOST_A:
                        if a_tile is None or job_ptr >= a_base + AC:
                            a_base = job_ptr
                            nb = min(AC, njobs - a_base)
                            a_tile = abp.tile([128, AC, 128], f8, tag="ab")
                            nc.sync.dma_start(a_tile[:, :nb, :],
                                              ahost_d[:, a_base:a_base + nb, :])
                    elif a_tile is None or job_ptr >= a_base + JB:
                        a_base = job_ptr
                        nb = min(JB, njobs - a_base)
                        a_tile = abp.tile([128, JB, 128], bf16, tag="ab")
                        nc.vector.tensor_tensor(
                            a_tile[:, :nb, :],
                            bagrel_s[:, a_base:a_base + nb, None]
                            .to_broadcast([128, nb, 128]),
                            iota_s[:, None, :].to_broadcast([128, nb, 128]),
                            op=ALU.is_equal,
                        )
                    nc.tensor.matmul(pooled[:, w, :],
                                     a_tile[:, job_ptr - a_base, :],
                                     gbf[:, i, :],
                                     start=st, stop=en, skip_group_check=True)
                    job_ptr += 1
            assert job_ptr == len(jobs)
            if pooled is not None:
                nc.scalar.activation(tall[:, :, cur_t + 1, :], pooled[:], AF.Copy)
                emit_interaction(cur_t + 1)

            # ---- top MLP (R^T tiles share the gather pool slots)
            rt = []
            for f in range(4):
                rt.append(gp.tile([128, 1024], f32, tag="gbuf", name=f"rt{f}"))
            nc.vector.memset(rt[3][:, :], 0.0)
            for f in range(4):
                wdt = 128 if f < 3 else 32
                for blk in range(NWIN):
                    ps = tpp.tile([128, 128], f32, tag="tp")
                    nc.tensor.transpose(ps[0:wdt, :],
                                        r_all[:, blk, f * 128:f * 128 + wdt],
                                        ident)
                    nc.vector.tensor_copy(
                        rt[f][0:wdt, blk * 128:(blk + 1) * 128], ps[0:wdt, :])
            for m in range(4):
                for n2 in range(2):
                    ps = mp.tile([128, 512], f32, tag="mp")
                    for kk in range(4):
                        nc.tensor.matmul(ps, tw1t_s[:, kk, m * 128:(m + 1) * 128],
                                         rt[kk][:, n2 * 512:(n2 + 1) * 512],
                                         start=(kk == 0), stop=(kk == 3))
                    nc.scalar.activation(h1t[:, m, n2 * 512:(n2 + 1) * 512], ps,
                                         AF.Relu, bias=tb1_s[:, m:m + 1])
            for m in range(2):
                for n2 in range(2):
                    ps = mp.tile([128, 512], f32, tag="mp")
                    for kk in range(4):
                        nc.tensor.matmul(ps, tw2t_s[:, kk, m * 128:(m + 1) * 128],
                                         h1t[:, kk, n2 * 512:(n2 + 1) * 512],
                                         start=(kk == 0), stop=(kk == 3))
                    nc.scalar.activation(h2t[:, m, n2 * 512:(n2 + 1) * 512], ps,
                                         AF.Relu, bias=tb2_s[:, m:m + 1])
            for n2 in range(2):
                ps = mp.tile([128, 512], f32, tag="mp")
                for kk in range(2):
                    nc.tensor.matmul(ps[0:1, :], tw3t_s[:, kk, :],
                                     h2t[:, kk, n2 * 512:(n2 + 1) * 512],
                                     start=(kk == 0), stop=(kk == 1))
                nc.scalar.activation(p_s[:, n2 * 512:(n2 + 1) * 512], ps[0:1, :],
                                     AF.Sigmoid, bias=tb3_s)
            nc.sync.dma_start(out_d.rearrange("a b -> b a"), p_s)

    nc.compile()
    return nc


# ---------------------------------------------------------------- ntff shim
def _install_ntff_shim():
    """Provide antenv.axon_hooks so run_bass_kernel_spmd(trace=True) can pull
    NTFF profiles through libaxon_pjrt (module is absent in this image)."""
    import types
    if "antenv.axon_hooks" in sys.modules:
        return
    try:
        import antenv
        from trn_agent_boot.trn_boot import _ntff_profile_via_ctypes
    except Exception:
        return
    mod = types.ModuleType("antenv.axon_hooks")
    _state = {"hook": None}
    mod.set_axon_ntff_profile_hook = lambda h: _state.__setitem__("hook", h)
    mod.get_axon_ntff_profile_hook = lambda: _state["hook"]
    sys.modules["antenv.axon_hooks"] = mod
    antenv.axon_hooks = mod
    try:
        hook = _ntff_profile_via_ctypes("/opt/axon/libaxon_pjrt.so")
        mod.set_axon_ntff_profile_hook(hook)
    except Exception:
        pass


# ---------------------------------------------------------------- entry
def kernel(x, lS_i, lS_o, emb_tables,
           bot_W1, bot_b1, bot_W2, bot_b2, bot_W3, bot_b3,
           top_W1, top_b1, top_W2, top_b2, top_W3, top_b3):
    t0 = time.time()
    static, idx16_rep, bagrel, nreal, ahost = _prep(lS_i)
    t1 = time.time()

    nc = _build(static)
    t2 = time.time()

    tabf = np.ascontiguousarray(np.asarray(emb_tables, dtype=np.float32))
    xT = np.ascontiguousarray(np.asarray(x, np.float32).T)          # [13, B]
    iota = np.tile(np.arange(128, dtype=np.float16), (128, 1))
    w1t = np.ascontiguousarray(np.asarray(bot_W1, np.float32).T)    # [13, 512]
    w2t = np.ascontiguousarray(
        np.asarray(bot_W2, np.float32).T.reshape(4, 128, 256).transpose(1, 0, 2))
    w3t = np.ascontiguousarray(
        np.asarray(bot_W3, np.float32).T.reshape(2, 128, 64).transpose(1, 0, 2))
    b1 = np.ascontiguousarray(np.asarray(bot_b1, np.float32).reshape(4, 128).T)
    b2 = np.ascontiguousarray(np.asarray(bot_b2, np.float32).reshape(2, 128).T)
    b3 = np.asarray(bot_b3, np.float32).reshape(64, 1)
    tw1 = np.zeros((512, 512), np.float32)
    tw1[:415, :] = np.asarray(top_W1, np.float32).T
    tw1t = np.ascontiguousarray(tw1.reshape(4, 128, 512).transpose(1, 0, 2))
    tw2t = np.ascontiguousarray(
        np.asarray(top_W2, np.float32).T.reshape(4, 128, 256).transpose(1, 0, 2))
    tw3t = np.ascontiguousarray(
        np.asarray(top_W3, np.float32).T.reshape(2, 128, 1).transpose(1, 0, 2))
    tb1 = np.ascontiguousarray(np.asarray(top_b1, np.float32).reshape(4, 128).T)
    tb2 = np.ascontiguousarray(np.asarray(top_b2, np.float32).reshape(2, 128).T)
    tb3 = np.asarray(top_b3, np.float32).reshape(1, 1)

    in_maps = []
    for c in range(NCORE):
        m = {
            "tab": tabf,
            "idx16": np.ascontiguousarray(idx16_rep[c]),
            "xT": np.ascontiguousarray(xT[:, c * BC:(c + 1) * BC]),
            "w1t": w1t, "w2t": w2t, "w3t": w3t,
            "b1": b1, "b2": b2, "b3": b3,
            "tw1t": tw1t, "tw2t": tw2t, "tw3t": tw3t,
            "tb1": tb1, "tb2": tb2, "tb3": tb3,
        }
        if HOST_A:
            m["ahost"] = np.ascontiguousarray(ahost[c])
        else:
            m["bagrel"] = np.ascontiguousarray(bagrel[c])
            m["iotaf"] = iota
        if NEGPAD:
            m["nreal"] = np.ascontiguousarray(nreal[c].reshape(1, -1))
        in_maps.append(m)

    from concourse.bass_utils import run_bass_kernel_spmd
    do_trace = bool(os.environ.get("DLRM_TRACE"))
    if do_trace:
        _install_ntff_shim()
    res = run_bass_kernel_spmd(nc, in_maps, core_ids=list(range(NCORE)),
                               trace=do_trace)
    t3 = time.time()
    if do_trace:
        with open("/tmp/dlrm_exec_ns.txt", "w") as f:
            f.write(str(res.exec_time_ns))
    print(f"[kernel] prep {t1-t0:.1f}s build+compile {t2-t1:.1f}s "
          f"run {t3-t2:.1f}s exec_ns={res.exec_time_ns}", file=sys.stderr)
    out = np.concatenate([r["out"] for r in res.results], axis=0)
    return out.astype(np.float32)

